# revision 65
# baseline (speedup 1.0000x reference)
"""Bass/Trainium2 kernel for GruAttCosMeanNet (nn_GruAttCosMeanNet_39591008535146).

Data-parallel over batch: 8 cores x 2 batch rows each.
Per core: bidirectional GRU encoders (context len 128, 5 options len 64),
Bahdanau additive attention per option, attention GRUs over the aggregated
sequences, cosine similarity.  Final softmax over 5 options is done on host
(16x5, negligible).

Device layouts (per core, p = SBUF partition):
  - GRU state/gates: [3H on partitions as 6 tiles of 128, batch cols on free]
  - recurrence matmul: stationary = Wh^T k-tile (bf16, FWL), moving = h cols
  - encoder outputs stored transposed [h-dim part, (t, col)] in bf16
  - attention energies: s[h, (q,c)] = tanh(optq + ctxk) built with
    broadcast APs on VE, tanh on SE, then e[c,q] via PE with s as stationary
    and v as the 1-column moving operand.
"""
import sys
sys.path.insert(0, "/opt/trn_rl_repo")
import numpy as np
import ml_dtypes

import concourse.bass as bass
import concourse.mybir as mybir
import concourse.tile as tile
from concourse import bacc, bass_utils
from concourse.masks import make_identity

BF16 = mybir.dt.bfloat16
F16 = mybir.dt.float16
F32 = mybir.dt.float32
AF = mybir.ActivationFunctionType
ALU = mybir.AluOpType

B, LC, LO, NOPT, E, H = 16, 128, 64, 5, 300, 256
NCORES = 8
BL = B // NCORES          # 2 batch rows per core
NI = BL * NOPT            # 10 (b,opt) pairs per core
NBM = BL + NI             # 12 cols in main GRU (2 ctx + 10 opt)
NBA = 2 * NI              # 20 cols in att GRU (10 actx + 10 aopt)
H3 = 3 * H                # 768
bf = ml_dtypes.bfloat16

_CACHE = {}


def _build():
    nc = bacc.Bacc("TRN2", target_bir_lowering=False, debug=False,
                   num_devices=NCORES)

    d = {}
    d["xtc"] = nc.dram_tensor("xtc", [3, 128, LC * BL], BF16, kind="ExternalInput")
    d["xto"] = nc.dram_tensor("xto", [3, 128, LO * NI], BF16, kind="ExternalInput")
    d["wir"] = nc.dram_tensor("wir", [2, 3, 128, H3], BF16, kind="ExternalInput")
    d["whr"] = nc.dram_tensor("whr", [2, 2, 128, H3], BF16, kind="ExternalInput")
    d["wia"] = nc.dram_tensor("wia", [2, 3, 128, H3], BF16, kind="ExternalInput")
    d["wha"] = nc.dram_tensor("wha", [2, 2, 128, H3], BF16, kind="ExternalInput")
    d["wk"] = nc.dram_tensor("wk", [4, 128, H], BF16, kind="ExternalInput")
    d["wq"] = nc.dram_tensor("wq", [4, 128, H], BF16, kind="ExternalInput")
    d["bhnrow"] = nc.dram_tensor("bhnrow", [2, 2, 2, 128], BF16,
                                 kind="ExternalInput")
    d["biasa"] = nc.dram_tensor("biasa", [128, 2, 6], F32,
                                kind="ExternalInput")
    d["v"] = nc.dram_tensor("v", [128, 2], F16, kind="ExternalInput")
    d["out"] = nc.dram_tensor("out", [1, NI], F32, kind="ExternalOutput")

    with tile.TileContext(nc) as tc:
        _body(nc, tc, d)
    nc.compile()
    return nc


def _body(nc, tc, d):
    import contextlib
    ctx = contextlib.ExitStack()
    with ctx:
        consts = ctx.enter_context(tc.tile_pool(name="consts", bufs=1))
        wpool = ctx.enter_context(tc.tile_pool(name="weights", bufs=1))
        xppool = ctx.enter_context(tc.tile_pool(name="xp", bufs=1))
        encp = ctx.enter_context(tc.tile_pool(name="enc", bufs=1))
        hpool = ctx.enter_context(tc.tile_pool(name="hstate", bufs=1))
        spool = ctx.enter_context(tc.tile_pool(name="spool", bufs=2))
        small = ctx.enter_context(tc.tile_pool(name="small", bufs=3))
        psg = ctx.enter_context(tc.tile_pool(name="psg", bufs=2, space="PSUM"))
        psum_hp = ctx.enter_context(tc.tile_pool(name="pshp", bufs=2, space="PSUM"))
        psum_e = ctx.enter_context(tc.tile_pool(name="pse", bufs=2, space="PSUM"))

        def ps_tile(shape):
            return psg.tile(shape, F32, tag="ps", name="pst")

        # ---- constants / weights ----
        ident = consts.tile([128, 128], BF16)
        make_identity(nc, ident[:])
        ones128 = consts.tile([128, 1], F32)
        nc.vector.memset(ones128[:], 1.0)
        ones_bf = consts.tile([128, 128], BF16)
        nc.vector.memset(ones_bf[:], 1.0)

        wir = wpool.tile([128, 2, 3, H3], BF16)
        whr = wpool.tile([128, 2, 2, H3], BF16)
        wia = wpool.tile([128, 2, 3, H3], BF16)
        wha = wpool.tile([128, 2, 2, H3], BF16)
        wk = wpool.tile([128, 4, H], BF16)
        wq = wpool.tile([128, 4, H], BF16)
        bhrow = consts.tile([1, 2, 2, 2, 128], BF16)
        ones_row = consts.tile([1, NBA], BF16)
        nc.vector.memset(ones_row[:], 1.0)
        vsb = consts.tile([128, 2], F16)
        # DMA order: what phase 1 and the main GRU need comes first
        xtc = wpool.tile([128, 3, LC * BL], BF16)
        xto = wpool.tile([128, 3, LO * NI], BF16)
        for k in range(3):
            nc.sync.dma_start(xtc[:, k, :], d["xtc"].ap()[k])
            nc.sync.dma_start(xto[:, k, :], d["xto"].ap()[k])
        for dd in range(2):
            for k in range(3):
                nc.sync.dma_start(wir[:, dd, k, :], d["wir"].ap()[dd, k])
            for k in range(2):
                nc.sync.dma_start(whr[:, dd, k, :], d["whr"].ap()[dd, k])
        nc.sync.dma_start(bhrow[0:1], d["bhnrow"].ap())
        for dd in range(2):
            for k in range(3):
                nc.sync.dma_start(wia[:, dd, k, :], d["wia"].ap()[dd, k])
            for k in range(2):
                nc.sync.dma_start(wha[:, dd, k, :], d["wha"].ap()[dd, k])
        for k in range(4):
            nc.sync.dma_start(wk[:, k, :], d["wk"].ap()[k])
            nc.sync.dma_start(wq[:, k, :], d["wq"].ap()[k])
        biasa = consts.tile([128, 2, 6], F32)
        nc.sync.dma_start(biasa[:], d["biasa"].ap())
        nc.sync.dma_start(vsb[:], d["v"].ap())

        # ======== Phase 1: main GRU input projections ========
        # One [.., LC, NBA]-wide tile is shared by both GRU phases: the main
        # GRU uses cols [0:NBM) (2 ctx + 10 opt), the att GRU later reuses
        # the full NBA cols (10 actx + 10 aopt).  Layout per phase:
        # [p, dir, gate, t, col]; short-seq cols are zero outside their
        # valid range; the dir=1 short-seq block sits at t in [64,128) so
        # the uniform bwd index T-1-t_f reads its time 63-t_f.
        xpu = xppool.tile([128, 2, LC, 6, NBA], BF16, tag="xpu")
        nc.vector.memset(xpu[:, 0, LO:, :, BL:NBM], 0.0)
        nc.vector.memset(xpu[:, 1, :LO, :, BL:NBM], 0.0)

        def proj_main(groups):
            for (xsrc, dd, tb, cl, ch, T2, nbg, tch) in groups:
                for jg in range(6):
                    for t0 in range(0, T2, tch):
                        tw = min(tch, T2 - t0)
                        cw = tw * nbg
                        pt = ps_tile([128, 512])
                        for k in range(3):
                            nc.tensor.matmul(
                                pt[:, :cw],
                                wir[:, dd, k, jg * 128:(jg + 1) * 128],
                                xsrc[:, k, t0 * nbg:t0 * nbg + cw],
                                start=(k == 0), stop=(k == 2))
                        if jg % 2 == 0:
                            nc.scalar.copy(
                                xpu[:, dd, tb + t0:tb + t0 + tw, jg, cl:ch],
                                pt[:, :cw])
                        else:
                            nc.vector.tensor_copy(
                                xpu[:, dd, tb + t0:tb + t0 + tw, jg, cl:ch],
                                pt[:, :cw])

        # NOTE: both directions project from the SAME (unreversed) input; the
        # bwd recurrence consumes xp at index Tb-1-t_f, which walks original
        # time in reverse — the true bwd GRU order.
        proj_main([
            (xtc, 0, 0, 0, BL, LC, BL, 128),
            (xtc, 1, 0, 0, BL, LC, BL, 128),
            (xto, 0, 0, BL, NBM, LO, NI, 32),
            (xto, 1, LO, BL, NBM, LO, NI, 32),
        ])

        # ======== Phase 2/6 shared: one bidirectional GRU time step ========
        # Per dir: hp = Wh @ h (+ bhn folded in as a 1-row PE matmul), then
        # VE: rz-add, nt-mult, nt-add; SE: sigmoid/tanh; GpSimd: the 3-op
        # h-update chain (engine balance: VE is the recurrence pacer).
        # Per-dir chains; the period of a GRU phase is the single-chain
        # latency, so the design minimizes critical-path ops + engine hops:
        # xp for gates r,z is PRE-ADDED into the PSUM via identity-matmuls
        # (hst-independent, so PE does them while waiting on the previous
        # step's h), sigmoid reads PSUM directly, and the whole post-tanh
        # update chain stays on VE (no extra engine hops).  Stores go to SE.
        # Critical chain per dir-step: PE(xp-preadd+Wh matmuls, contiguous
        # per accumulation group) -> SE sigmoid (reads PSUM) -> VE nt ops ->
        # SE tanh -> VE 2-op tail.  zbar=1-z and z*h are precomputed on the
        # idle Pool engine off the critical path: h' = zbar*n + z*h.
        def gru_step(t_f, whx, xpa, hst, nb, which, store):
            for dd in range(2):
                t2 = t_f if dd == 0 else LC - 1 - t_f
                hp = psum_hp.tile([128, 6, nb], F32, tag="hp")
                for jg in range(6):
                    if jg < 4:
                        nc.tensor.matmul(
                            hp[:, jg, :], ident[:, 0:128],
                            xpa[:, dd, t2, jg, 0:nb], start=True, stop=False)
                    else:
                        nc.tensor.matmul(
                            hp[:, jg, :], bhrow[0:1, which, dd, jg - 4, :],
                            ones_row[0:1, :nb], start=True, stop=False)
                    nc.tensor.matmul(
                        hp[:, jg, :], whx[:, dd, 0, jg * 128:(jg + 1) * 128],
                        hst[:, dd, 0, :], start=False, stop=False)
                    nc.tensor.matmul(
                        hp[:, jg, :], whx[:, dd, 1, jg * 128:(jg + 1) * 128],
                        hst[:, dd, 1, :], start=False, stop=True)
                rz = small.tile([128, 4, nb], F32, tag=f"rz{dd}")
                nc.scalar.activation(rz[:], hp[:, 0:4, :], AF.Sigmoid)
                zb = small.tile([128, 2, nb], F32, tag=f"zb{dd}")
                nc.gpsimd.tensor_scalar(zb[:], rz[:, 2:4, :], 1.0, -1.0,
                                        op0=ALU.subtract, op1=ALU.mult)
                zh = small.tile([128, 2, nb], F32, tag=f"zh{dd}")
                nc.gpsimd.tensor_tensor(zh[:], rz[:, 2:4, :],
                                        hst[:, dd, :, :], ALU.mult)
                nt = small.tile([128, 2, nb], F32, tag=f"nt{dd}")
                nc.vector.tensor_tensor(nt[:], rz[:, 0:2, :], hp[:, 4:6, :],
                                        ALU.mult)
                nc.vector.tensor_tensor(nt[:], nt[:],
                                        xpa[:, dd, t2, 4:6, 0:nb], ALU.add)
                nc.scalar.activation(nt[:], nt[:], AF.Tanh)
                nc.vector.tensor_tensor(nt[:], zb[:], nt[:], ALU.mult)
                nc.vector.tensor_tensor(hst[:, dd, :, :], nt[:], zh[:],
                                        ALU.add)
                store(dd, t_f, hst)

        # ======== Phase 2: main GRU recurrence ========
        # enc: [p, dir, jg, t, col]; ctx cols [0:BL) valid for all t, opt
        # cols [BL:NBM) valid for t in [0,64) (both dirs store the opt state
        # at its own output position).
        enc = encp.tile([128, 2, LC, 2, NBM], BF16)
        hm = hpool.tile([128, 2, 2, NBM], BF16, tag="h")
        nc.vector.memset(hm[:], 0.0)

        def store_main(dd, t_f, hst):
            if dd == 0:
                if t_f < LO:
                    nc.vector.tensor_copy(enc[:, 0, t_f, :, :],
                                          hst[:, 0, :, 0:NBM])
                else:
                    nc.vector.tensor_copy(enc[:, 0, t_f, :, 0:BL],
                                          hst[:, 0, :, 0:BL])
            else:
                nc.vector.tensor_copy(enc[:, 1, LC - 1 - t_f, :, 0:BL],
                                      hst[:, 1, :, 0:BL])
                if t_f < LO:
                    nc.vector.tensor_copy(enc[:, 1, LO - 1 - t_f, :, BL:],
                                          hst[:, 1, :, BL:NBM])

        # ======== Phase 3 tiles (emitted interleaved into phase 2) ========
        ctxkT = encp.tile([128, 2, LC, BL], BF16)
        optqT = encp.tile([128, 2, LO, NI], BF16)

        def kq_chunk(dst, w, jg, t0, tw, nb2, cl, ch):
            cw = tw * nb2
            pt = ps_tile([128, 512])
            for k in range(4):
                nc.tensor.matmul(
                    pt[:, :cw], w[:, k, jg * 128:(jg + 1) * 128],
                    enc[:, k // 2, t0:t0 + tw, k % 2, cl:ch],
                    start=(k == 0), stop=(k == 3))
            nc.vector.tensor_copy(dst[:, jg, t0:t0 + tw, :], pt[:, :cw])

        def bf_transpose(dst, src, pcols, ocols):
            """src [pcols, ocols] bf16 -> dst [ocols, pcols] bf16."""
            pt = psg.tile([128, 512], BF16, tag="psbf", name="pst_bf")
            nc.tensor.transpose(pt[:ocols, :pcols], src, ident[:pcols, :pcols])
            nc.vector.tensor_copy(dst, pt[:ocols, :pcols])

        ck_t = encp.tile([128, BL, 2, 128], BF16)
        oq_t = encp.tile([64, NI, 2, 128], BF16)

        # the opt half of enc is complete after step 63, so opt_q projection
        # and the oq transposes drain into the PE-idle slots of main-GRU
        # steps 64..127 (one thunk per step, emitted inline)
        pending = []
        for jg in range(2):
            for t0 in range(0, LO, 32):
                pending.append(lambda jg=jg, t0=t0: kq_chunk(
                    optqT, wq, jg, t0, min(32, LO - t0), NI, BL, NBM))
        for i in range(NI):
            for jg in range(2):
                pending.append(lambda i=i, jg=jg: bf_transpose(
                    oq_t[:, i, jg, :], optqT[:, jg, :, i], 128, 64))

        for t in range(LC):
            gru_step(t, whr, xpu, hm, NBM, 0, store_main)
            if t >= LO and pending:
                pending.pop(0)()
        while pending:
            pending.pop(0)()

        # ctx_key needs the full context encoding
        for jg in range(2):
            kq_chunk(ctxkT, wk, jg, 0, LC, BL, 0, BL)
        for b in range(BL):
            for jg in range(2):
                bf_transpose(ck_t[:, b, jg, :], ctxkT[:, jg, :, b], 128, 128)
        ctxk_cb = [[ck_t[:, b, jg, :] for jg in range(2)] for b in range(BL)]
        oq_all = [[oq_t[:, i, jg, :] for jg in range(2)] for i in range(NI)]

        # ======== Phase 4: attention per (b, opt) ========
        # |e| <= sum|v| ~ 8, so exp() is safe in fp32 without max-subtraction.
        # Both softmaxes share one exp(e): P1 = eu/rowsum (softmax over q,
        # free axis), P2 = eu/colsum (softmax over c, partition axis; the
        # colsum comes from a ones-row matmul and normalization is folded
        # into the aggregation post-scale).
        actxT = encp.tile([128, 2, NI, LC], BF16)
        aoptT = encp.tile([128, 2, NI, LO], BF16)
        QCH = 32
        for b in range(BL):
            for o in range(NOPT):
                i = b * NOPT + o
                e_ps = psum_e.tile([128, LO], F32, tag="e")
                for q0 in range(0, LO, QCH):
                    sts = []
                    for jg in range(2):
                        # all-16-bit s-build (DVE 2x perf mode); fp16 keeps
                        # the energy quantization error small
                        st = spool.tile([128, QCH, LC], F16, tag=f"s{jg}")
                        eng = nc.vector if (q0 // QCH + jg) % 2 == 0 else nc.gpsimd
                        eng.tensor_tensor(
                            st[:],
                            optqT[:, jg, q0:q0 + QCH, i:i + 1]
                                .broadcast_to([128, QCH, LC]),
                            ctxkT[:, jg, None, :, b]
                                .broadcast_to([128, QCH, LC]),
                            ALU.add)
                        nc.scalar.activation(st[:], st[:], AF.Tanh)
                        sts.append(st)
                    for q in range(QCH):
                        for jg in range(2):
                            nc.tensor.matmul(
                                e_ps[:, q0 + q:q0 + q + 1],
                                sts[jg][:, q, :], vsb[:, jg:jg + 1],
                                start=(jg == 0), stop=(jg == 1))
                # shared unnormalized exp(e) [c, q]
                eu = small.tile([128, LO], BF16, tag="eu")
                nc.scalar.activation(eu[:], e_ps[:], AF.Exp)
                # P1: softmax over q (free axis)
                sm = small.tile([128, 1], F32, tag="sm")
                nc.vector.tensor_reduce(sm[:], eu[:],
                                        axis=mybir.AxisListType.X, op=ALU.add)
                nc.vector.reciprocal(sm[:], sm[:])
                p1 = small.tile([128, LO], BF16, tag="p1")
                nc.vector.tensor_scalar_mul(p1[:], eu[:], sm[:])
                pt1 = psg.tile([128, 512], BF16, tag="psbf", name="pst_bf")
                nc.tensor.transpose(pt1[:64, :128], p1[:], ident[:])
                p1t = small.tile([64, 128], BF16, tag="p1tb")
                nc.vector.tensor_copy(p1t[:], pt1[:64, :128])
                # P2 colsum over c, replicated across partitions by a
                # ones-matrix matmul; normalize after aggregation
                s2_ps = ps_tile([128, 512])
                nc.tensor.matmul(s2_ps[:, :LO], ones_bf[:], eu[:],
                                 start=True, stop=True)
                r2 = small.tile([128, LO], F32, tag="r2")
                nc.vector.reciprocal(r2[:], s2_ps[:, :LO])
                for jg in range(2):
                    ac_ps = ps_tile([128, 512])
                    nc.tensor.matmul(ac_ps[:, :128], oq_all[i][jg], p1t[:],
                                     start=True, stop=True)
                    nc.scalar.copy(actxT[:, jg, i, :], ac_ps[:, :128])
                    ao_ps = ps_tile([128, 512])
                    nc.tensor.matmul(ao_ps[:, :64], ctxk_cb[b][jg], eu[:],
                                     start=True, stop=True)
                    nc.vector.tensor_tensor(
                        aoptT[:, jg, i, :], ao_ps[:, :64], r2[:], ALU.mult)

        # ======== Phase 5: att GRU input projections ========
        # Reuses xpu (main-GRU xp is dead): cols [0:NI) = actx (valid all t,
        # both dirs), cols [NI:NBA) = aopt (dir0 at t in [0,64), dir1 at t in
        # [64,128), so the uniform bwd index 127-t_f reads aopt time 63-t_f).
        # The xp bias is applied by the psum->sbuf copy (per-partition bias
        # add), not a matmul; copies round-robin across SE/VE/Pool.
        nc.gpsimd.memset(xpu[:, 0, LO:, :, NI:], 0.0)
        nc.gpsimd.memset(xpu[:, 1, :LO, :, NI:], 0.0)
        cp_eng = [0]

        def copy_bias(dst, src, bias_ap):
            e = cp_eng[0] = (cp_eng[0] + 1) % 2
            if e == 0:
                nc.scalar.activation(dst, src, AF.Identity, bias=bias_ap)
            else:
                nc.vector.tensor_scalar(dst, src, bias_ap, None, op0=ALU.add)

        def proj_att_chunk(src, dd, jg, t0, tch, T, cl, ch, tb1):
            tb = tb1 if dd == 1 else 0
            tw = min(tch, T - t0)
            cw = tw * NI
            pt = ps_tile([128, 512])
            for k in range(2):
                nc.tensor.matmul(
                    pt[:, :cw],
                    wia[:, dd, k, jg * 128:(jg + 1) * 128],
                    src[:, k, t0:t0 + tw, :],
                    start=(k == 0), stop=(k == 1))
            copy_bias(
                xpu[:, dd, tb + t0:tb + t0 + tw, jg, cl:ch],
                pt[:, :cw], biasa[:, dd, jg:jg + 1])

        # transposed views [128, k, t, i] of actxT/aoptT ([128, jg, i, t])
        acv = actxT[:].transpose([0, 1, 3, 2])
        aov = aoptT[:].transpose([0, 1, 3, 2])
        # outside-in: steps 0..31 of the att GRU only touch t in [0,32) and
        # [96,128), so the aopt and end actx chunks are projected up front
        # while the middle actx chunks drain into the first GRU steps
        for t0 in (0, 32):
            for dd in range(2):
                for jg in range(6):
                    proj_att_chunk(aov, dd, jg, t0, 32, LO, NI, NBA, LO)
        for t0 in (0, 96):
            for dd in range(2):
                for jg in range(6):
                    proj_att_chunk(acv, dd, jg, t0, 32, LC, 0, NI, 0)
        pending2 = []
        for t0 in (32, 64):
            for dd in range(2):
                for jg in range(6):
                    pending2.append(lambda t0=t0, dd=dd, jg=jg: proj_att_chunk(
                        acv, dd, jg, t0, 32, LC, 0, NI, 0))

        # ======== Phase 6: att GRU recurrence with mean accumulation ========
        ha = hpool.tile([128, 2, 2, NBA], BF16, tag="h")
        nc.vector.memset(ha[:], 0.0)
        acc = encp.tile([128, 2, 2, NBA], F32)
        nc.vector.memset(acc[:], 0.0)

        def store_att(dd, t_f, hst):
            if t_f < LO:
                nc.vector.tensor_tensor(acc[:, dd], acc[:, dd],
                                        hst[:, dd, :, :], ALU.add)
            else:
                nc.vector.tensor_tensor(acc[:, dd, :, 0:NI],
                                        acc[:, dd, :, 0:NI],
                                        hst[:, dd, :, 0:NI], ALU.add)

        for t in range(LC):
            gru_step(t, wha, xpu, ha, NBA, 1, store_att)
            if pending2:
                pending2.pop(0)()

        # ======== Phase 7: cosine similarity ========
        # means: acc cols [0:NI) = a_ctx side (per 1/LC), [NI:) = a_opt side
        # (per 1/LO); the 1/LC / 1/LO scales cancel in the cosine except as
        # a joint scale, but keep them for exactness.
        acc_c = acc[:, :, :, 0:NI]
        acc_o = acc[:, :, :, NI:NBA]
        nc.vector.tensor_scalar_mul(acc_c, acc_c, 1.0 / LC)
        nc.vector.tensor_scalar_mul(acc_o, acc_o, 1.0 / LO)
        prod = small.tile([128, 2, 2, NI], F32, tag="prod")
        dots_ps = psg.tile([1, 3, 4, NI], F32, tag="ps")
        nc.vector.tensor_tensor(prod[:], acc_c, acc_o, ALU.mult)
        nc.tensor.matmul(dots_ps[:, 0], ones128[:], prod[:],
                         start=True, stop=True)
        nc.vector.tensor_tensor(prod[:], acc_c, acc_c, ALU.mult)
        nc.tensor.matmul(dots_ps[:, 1], ones128[:], prod[:],
                         start=True, stop=True)
        nc.vector.tensor_tensor(prod[:], acc_o, acc_o, ALU.mult)
        nc.tensor.matmul(dots_ps[:, 2], ones128[:], prod[:],
                         start=True, stop=True)
        red = small.tile([1, 3, NI], F32, tag="red")
        nc.vector.tensor_reduce(red[:], dots_ps[:].transpose([0, 1, 3, 2]),
                                axis=mybir.AxisListType.X, op=ALU.add)
        nrm = small.tile([1, NI], F32, tag="nrm")
        nc.vector.tensor_tensor(nrm[:], red[:, 1, :], red[:, 2, :], ALU.mult)
        nc.vector.tensor_scalar_max(nrm[:], nrm[:], 1e-30)
        nc.scalar.activation(nrm[:], nrm[:], AF.Sqrt)
        nc.vector.reciprocal(nrm[:], nrm[:])
        cos = small.tile([1, NI], F32, tag="cos")
        nc.vector.tensor_tensor(cos[:], red[:, 0, :], nrm[:], ALU.mult)
        nc.sync.dma_start(d["out"].ap(), cos[:])


def _prep_inputs(inputs):
    ctx = np.asarray(inputs["context"], np.float32)
    opts = np.asarray(inputs["options"], np.float32)

    def gru_w(pre):
        out = {}
        for dd, sfx in enumerate(("f", "b")):
            out[dd] = {k: np.asarray(inputs[f"{pre}_{k}_{sfx}"], np.float32)
                       for k in ("Wi", "Wh", "bi", "bh")}
        return out

    rnn, att = gru_w("rnn"), gru_w("att")
    Wk = np.asarray(inputs["Wk"], np.float32)
    Wq = np.asarray(inputs["Wq"], np.float32)
    v = np.asarray(inputs["v_energy"], np.float32)

    def wi_pack(g, ein):
        out = np.zeros((2, 3, 128, H3), np.float32)
        for dd in range(2):
            bias = g[dd]["bi"].copy()
            bias[:2 * H] += g[dd]["bh"][:2 * H]
            m = np.zeros((3 * 128, H3), np.float32)
            m[:ein] = g[dd]["Wi"].T
            m[ein] = bias
            out[dd] = m.reshape(3, 128, H3)
        return out.astype(bf)

    def wh_pack(g):
        out = np.zeros((2, 2, 128, H3), np.float32)
        for dd in range(2):
            out[dd] = g[dd]["Wh"].T.reshape(2, 128, H3)
        return out.astype(bf)

    def bhn_pack(g):
        out = np.zeros((2, 2, 128), np.float32)
        for dd in range(2):
            out[dd, 0] = g[dd]["bh"][2 * H:2 * H + 128]
            out[dd, 1] = g[dd]["bh"][2 * H + 128:]
        return out

    def biasa_pack(g):
        # xp bias for the att GRU: bi with bh folded in for the r,z gates,
        # laid out [h-partition, dir, gate-block]
        out = np.zeros((128, 2, 6), np.float32)
        for dd in range(2):
            bias = g[dd]["bi"].copy()
            bias[:2 * H] += g[dd]["bh"][:2 * H]
            out[:, dd, :] = bias.reshape(6, 128).T
        return out

    shared = {
        "wir": wi_pack(rnn, E), "whr": wh_pack(rnn),
        "wia": wi_pack(att, H), "wha": wh_pack(att),
        "wk": np.ascontiguousarray(Wk.T.reshape(4, 128, H).astype(bf)),
        "wq": np.ascontiguousarray(Wq.T.reshape(4, 128, H).astype(bf)),
        "bhnrow": np.ascontiguousarray(
            np.stack([bhn_pack(rnn), bhn_pack(att)]).astype(bf)),
        "biasa": np.ascontiguousarray(biasa_pack(att)),
        "v": np.ascontiguousarray(v.reshape(2, 128).T.astype(np.float16)),
    }

    in_maps = []
    for c in range(NCORES):
        bs = slice(c * BL, (c + 1) * BL)
        xa = np.zeros((BL, LC, 3 * 128), np.float32)
        xa[:, :, :E] = ctx[bs]
        xa[:, :, E] = 1.0
        xb = np.zeros((NI, LO, 3 * 128), np.float32)
        xb[:, :, :E] = opts[bs].reshape(NI, LO, E)
        xb[:, :, E] = 1.0
        m = dict(shared)
        m["xtc"] = np.ascontiguousarray(
            xa.transpose(2, 1, 0).reshape(3, 128, LC * BL).astype(bf))
        m["xto"] = np.ascontiguousarray(
            xb.transpose(2, 1, 0).reshape(3, 128, LO * NI).astype(bf))
        in_maps.append(m)
    return in_maps


def kernel(**inputs):
    if "nc" not in _CACHE:
        _CACHE["nc"] = _build()
    nc = _CACHE["nc"]
    in_maps = _prep_inputs(inputs)
    res = bass_utils.run_bass_kernel_spmd(nc, in_maps,
                                          core_ids=list(range(NCORES)))
    _CACHE["last_exec_ns"] = res.exec_time_ns
    _CACHE["last_res"] = res
    logits = np.concatenate(
        [np.asarray(res.results[c]["out"], np.float32).reshape(BL, NOPT)
         for c in range(NCORES)], axis=0)
    x = logits - logits.max(axis=1, keepdims=True)
    ex = np.exp(x)
    return (ex / ex.sum(axis=1, keepdims=True)).astype(np.float32)


if __name__ == "__main__":
    _build()
    print("build+compile OK")



# revision 66
# speedup vs baseline: 1.0141x; 1.0141x over previous
"""Bass/Trainium2 kernel for GruAttCosMeanNet (nn_GruAttCosMeanNet_39591008535146).

Data-parallel over batch: 8 cores x 2 batch rows each.
Per core: bidirectional GRU encoders (context len 128, 5 options len 64),
Bahdanau additive attention per option, attention GRUs over the aggregated
sequences, cosine similarity.  Final softmax over 5 options is done on host
(16x5, negligible).

Device layouts (per core, p = SBUF partition):
  - GRU state/gates: [3H on partitions as 6 tiles of 128, batch cols on free]
  - recurrence matmul: stationary = Wh^T k-tile (bf16, FWL), moving = h cols
  - encoder outputs stored transposed [h-dim part, (t, col)] in bf16
  - attention energies: s[h, (q,c)] = tanh(optq + ctxk) built with
    broadcast APs on VE, tanh on SE, then e[c,q] via PE with s as stationary
    and v as the 1-column moving operand.
"""
import sys
sys.path.insert(0, "/opt/trn_rl_repo")
import numpy as np
import ml_dtypes

import concourse.bass as bass
import concourse.mybir as mybir
import concourse.tile as tile
from concourse import bacc, bass_utils
from concourse.masks import make_identity

BF16 = mybir.dt.bfloat16
F16 = mybir.dt.float16
F32 = mybir.dt.float32
AF = mybir.ActivationFunctionType
ALU = mybir.AluOpType

B, LC, LO, NOPT, E, H = 16, 128, 64, 5, 300, 256
NCORES = 8
BL = B // NCORES          # 2 batch rows per core
NI = BL * NOPT            # 10 (b,opt) pairs per core
NBM = BL + NI             # 12 cols in main GRU (2 ctx + 10 opt)
NBA = 2 * NI              # 20 cols in att GRU (10 actx + 10 aopt)
H3 = 3 * H                # 768
bf = ml_dtypes.bfloat16

_CACHE = {}


def _build():
    nc = bacc.Bacc("TRN2", target_bir_lowering=False, debug=False,
                   num_devices=NCORES)

    d = {}
    d["xtc"] = nc.dram_tensor("xtc", [3, 128, LC * BL], BF16, kind="ExternalInput")
    d["xto"] = nc.dram_tensor("xto", [3, 128, LO * NI], BF16, kind="ExternalInput")
    d["wir"] = nc.dram_tensor("wir", [2, 3, 128, H3], BF16, kind="ExternalInput")
    d["whr"] = nc.dram_tensor("whr", [2, 2, 128, H3], BF16, kind="ExternalInput")
    d["wia"] = nc.dram_tensor("wia", [2, 3, 128, H3], BF16, kind="ExternalInput")
    d["wha"] = nc.dram_tensor("wha", [2, 2, 128, H3], BF16, kind="ExternalInput")
    d["wk"] = nc.dram_tensor("wk", [4, 128, H], BF16, kind="ExternalInput")
    d["wq"] = nc.dram_tensor("wq", [4, 128, H], BF16, kind="ExternalInput")
    d["bhnrow"] = nc.dram_tensor("bhnrow", [2, 2, 2, 128], BF16,
                                 kind="ExternalInput")
    d["biasa"] = nc.dram_tensor("biasa", [128, 2, 6], F32,
                                kind="ExternalInput")
    d["v"] = nc.dram_tensor("v", [128, 2], F16, kind="ExternalInput")
    d["out"] = nc.dram_tensor("out", [1, NI], F32, kind="ExternalOutput")

    with tile.TileContext(nc) as tc:
        _body(nc, tc, d)
    nc.compile()
    return nc


def _body(nc, tc, d):
    import contextlib
    ctx = contextlib.ExitStack()
    with ctx:
        consts = ctx.enter_context(tc.tile_pool(name="consts", bufs=1))
        wpool = ctx.enter_context(tc.tile_pool(name="weights", bufs=1))
        xppool = ctx.enter_context(tc.tile_pool(name="xp", bufs=1))
        encp = ctx.enter_context(tc.tile_pool(name="enc", bufs=1))
        hpool = ctx.enter_context(tc.tile_pool(name="hstate", bufs=1))
        spool = ctx.enter_context(tc.tile_pool(name="spool", bufs=2))
        small = ctx.enter_context(tc.tile_pool(name="small", bufs=3))
        psg = ctx.enter_context(tc.tile_pool(name="psg", bufs=2, space="PSUM"))
        psum_hp = ctx.enter_context(tc.tile_pool(name="pshp", bufs=2, space="PSUM"))
        psum_e = ctx.enter_context(tc.tile_pool(name="pse", bufs=2, space="PSUM"))

        def ps_tile(shape):
            return psg.tile(shape, F32, tag="ps", name="pst")

        # ---- constants / weights ----
        ident = consts.tile([128, 128], BF16)
        make_identity(nc, ident[:])
        ones128 = consts.tile([128, 1], F32)
        nc.vector.memset(ones128[:], 1.0)
        ones_bf = consts.tile([128, 128], BF16)
        nc.vector.memset(ones_bf[:], 1.0)

        wir = wpool.tile([128, 2, 3, H3], BF16)
        whr = wpool.tile([128, 2, 2, H3], BF16)
        wia = wpool.tile([128, 2, 3, H3], BF16)
        wha = wpool.tile([128, 2, 2, H3], BF16)
        wk = wpool.tile([128, 4, H], BF16)
        wq = wpool.tile([128, 4, H], BF16)
        bhrow = consts.tile([1, 2, 2, 2, 128], BF16)
        ones_row = consts.tile([1, NBA], BF16)
        nc.vector.memset(ones_row[:], 1.0)
        vsb = consts.tile([128, 2], F16)
        # DMA order: what phase 1 and the main GRU need comes first
        xtc = wpool.tile([128, 3, LC * BL], BF16)
        xto = wpool.tile([128, 3, LO * NI], BF16)
        for k in range(3):
            nc.sync.dma_start(xtc[:, k, :], d["xtc"].ap()[k])
            nc.sync.dma_start(xto[:, k, :], d["xto"].ap()[k])
        for dd in range(2):
            for k in range(3):
                nc.sync.dma_start(wir[:, dd, k, :], d["wir"].ap()[dd, k])
            for k in range(2):
                nc.sync.dma_start(whr[:, dd, k, :], d["whr"].ap()[dd, k])
        nc.sync.dma_start(bhrow[0:1], d["bhnrow"].ap())
        for dd in range(2):
            for k in range(3):
                nc.sync.dma_start(wia[:, dd, k, :], d["wia"].ap()[dd, k])
            for k in range(2):
                nc.sync.dma_start(wha[:, dd, k, :], d["wha"].ap()[dd, k])
        for k in range(4):
            nc.sync.dma_start(wk[:, k, :], d["wk"].ap()[k])
            nc.sync.dma_start(wq[:, k, :], d["wq"].ap()[k])
        biasa = consts.tile([128, 2, 6], F32)
        nc.sync.dma_start(biasa[:], d["biasa"].ap())
        nc.sync.dma_start(vsb[:], d["v"].ap())

        # ======== Phase 1: main GRU input projections ========
        # One [.., LC, NBA]-wide tile is shared by both GRU phases: the main
        # GRU uses cols [0:NBM) (2 ctx + 10 opt), the att GRU later reuses
        # the full NBA cols (10 actx + 10 aopt).  Layout per phase:
        # [p, dir, gate, t, col]; short-seq cols are zero outside their
        # valid range; the dir=1 short-seq block sits at t in [64,128) so
        # the uniform bwd index T-1-t_f reads its time 63-t_f.
        xpu = xppool.tile([128, 2, LC, 6, NBA], BF16, tag="xpu")
        nc.vector.memset(xpu[:, 0, LO:, :, BL:NBM], 0.0)
        nc.vector.memset(xpu[:, 1, :LO, :, BL:NBM], 0.0)

        def proj_main(groups):
            for (xsrc, dd, tb, cl, ch, T2, nbg, tch) in groups:
                for jg in range(6):
                    for t0 in range(0, T2, tch):
                        tw = min(tch, T2 - t0)
                        cw = tw * nbg
                        pt = ps_tile([128, 512])
                        for k in range(3):
                            nc.tensor.matmul(
                                pt[:, :cw],
                                wir[:, dd, k, jg * 128:(jg + 1) * 128],
                                xsrc[:, k, t0 * nbg:t0 * nbg + cw],
                                start=(k == 0), stop=(k == 2))
                        if jg % 2 == 0:
                            nc.scalar.copy(
                                xpu[:, dd, tb + t0:tb + t0 + tw, jg, cl:ch],
                                pt[:, :cw])
                        else:
                            nc.vector.tensor_copy(
                                xpu[:, dd, tb + t0:tb + t0 + tw, jg, cl:ch],
                                pt[:, :cw])

        # NOTE: both directions project from the SAME (unreversed) input; the
        # bwd recurrence consumes xp at index Tb-1-t_f, which walks original
        # time in reverse — the true bwd GRU order.
        proj_main([
            (xtc, 0, 0, 0, BL, LC, BL, 128),
            (xtc, 1, 0, 0, BL, LC, BL, 128),
            (xto, 0, 0, BL, NBM, LO, NI, 32),
            (xto, 1, LO, BL, NBM, LO, NI, 32),
        ])

        # ======== Phase 2/6 shared: one bidirectional GRU time step ========
        # Per dir: hp = Wh @ h (+ bhn folded in as a 1-row PE matmul), then
        # VE: rz-add, nt-mult, nt-add; SE: sigmoid/tanh; GpSimd: the 3-op
        # h-update chain (engine balance: VE is the recurrence pacer).
        # Per-dir chains; the period of a GRU phase is the single-chain
        # latency, so the design minimizes critical-path ops + engine hops:
        # xp for gates r,z is PRE-ADDED into the PSUM via identity-matmuls
        # (hst-independent, so PE does them while waiting on the previous
        # step's h), sigmoid reads PSUM directly, and the whole post-tanh
        # update chain stays on VE (no extra engine hops).  Stores go to SE.
        # Critical chain per dir-step: PE(xp-preadd+Wh matmuls, contiguous
        # per accumulation group) -> SE sigmoid (reads PSUM) -> VE nt ops ->
        # SE tanh -> VE 2-op tail.  zbar=1-z and z*h are precomputed on the
        # idle Pool engine off the critical path: h' = zbar*n + z*h.
        def gru_step(t_f, whx, xpa, hst, nb, which, store):
            for dd in range(2):
                t2 = t_f if dd == 0 else LC - 1 - t_f
                hp = psum_hp.tile([128, 6, nb], F32, tag="hp")
                for jg in range(6):
                    if jg < 4:
                        nc.tensor.matmul(
                            hp[:, jg, :], ident[:, 0:128],
                            xpa[:, dd, t2, jg, 0:nb], start=True, stop=False)
                    else:
                        nc.tensor.matmul(
                            hp[:, jg, :], bhrow[0:1, which, dd, jg - 4, :],
                            ones_row[0:1, :nb], start=True, stop=False)
                    nc.tensor.matmul(
                        hp[:, jg, :], whx[:, dd, 0, jg * 128:(jg + 1) * 128],
                        hst[:, dd, 0, :], start=False, stop=False)
                    nc.tensor.matmul(
                        hp[:, jg, :], whx[:, dd, 1, jg * 128:(jg + 1) * 128],
                        hst[:, dd, 1, :], start=False, stop=True)
                rz = small.tile([128, 4, nb], F32, tag=f"rz{dd}")
                nc.scalar.activation(rz[:], hp[:, 0:4, :], AF.Sigmoid)
                zb = small.tile([128, 2, nb], F32, tag=f"zb{dd}")
                nc.gpsimd.tensor_scalar(zb[:], rz[:, 2:4, :], 1.0, -1.0,
                                        op0=ALU.subtract, op1=ALU.mult)
                zh = small.tile([128, 2, nb], F32, tag=f"zh{dd}")
                nc.gpsimd.tensor_tensor(zh[:], rz[:, 2:4, :],
                                        hst[:, dd, :, :], ALU.mult)
                nt = small.tile([128, 2, nb], F32, tag=f"nt{dd}")
                nc.vector.tensor_tensor(nt[:], rz[:, 0:2, :], hp[:, 4:6, :],
                                        ALU.mult)
                nc.vector.tensor_tensor(nt[:], nt[:],
                                        xpa[:, dd, t2, 4:6, 0:nb], ALU.add)
                nc.scalar.activation(nt[:], nt[:], AF.Tanh)
                nc.vector.tensor_tensor(nt[:], zb[:], nt[:], ALU.mult)
                nc.vector.tensor_tensor(hst[:, dd, :, :], nt[:], zh[:],
                                        ALU.add)
                store(dd, t_f, hst)

        # ======== Phase 2: main GRU recurrence ========
        # enc: [p, dir, jg, t, col]; ctx cols [0:BL) valid for all t, opt
        # cols [BL:NBM) valid for t in [0,64) (both dirs store the opt state
        # at its own output position).
        enc = encp.tile([128, 2, LC, 2, NBM], BF16)
        hm = hpool.tile([128, 2, 2, NBM], BF16, tag="h")
        nc.vector.memset(hm[:], 0.0)

        def store_main(dd, t_f, hst):
            if dd == 0:
                if t_f < LO:
                    nc.vector.tensor_copy(enc[:, 0, t_f, :, :],
                                          hst[:, 0, :, 0:NBM])
                else:
                    nc.vector.tensor_copy(enc[:, 0, t_f, :, 0:BL],
                                          hst[:, 0, :, 0:BL])
            else:
                nc.vector.tensor_copy(enc[:, 1, LC - 1 - t_f, :, 0:BL],
                                      hst[:, 1, :, 0:BL])
                if t_f < LO:
                    nc.vector.tensor_copy(enc[:, 1, LO - 1 - t_f, :, BL:],
                                          hst[:, 1, :, BL:NBM])

        # ======== Phase 3 tiles (emitted interleaved into phase 2) ========
        ctxkT = encp.tile([128, 2, LC, BL], BF16)
        optqT = encp.tile([128, 2, LO, NI], BF16)

        def kq_chunk(dst, w, jg, t0, tw, nb2, cl, ch):
            cw = tw * nb2
            pt = ps_tile([128, 512])
            for k in range(4):
                nc.tensor.matmul(
                    pt[:, :cw], w[:, k, jg * 128:(jg + 1) * 128],
                    enc[:, k // 2, t0:t0 + tw, k % 2, cl:ch],
                    start=(k == 0), stop=(k == 3))
            nc.vector.tensor_copy(dst[:, jg, t0:t0 + tw, :], pt[:, :cw])

        def bf_transpose(dst, src, pcols, ocols):
            """src [pcols, ocols] bf16 -> dst [ocols, pcols] bf16."""
            pt = psg.tile([128, 512], BF16, tag="psbf", name="pst_bf")
            nc.tensor.transpose(pt[:ocols, :pcols], src, ident[:pcols, :pcols])
            nc.vector.tensor_copy(dst, pt[:ocols, :pcols])

        ck_t = encp.tile([128, BL, 2, 128], BF16)
        oq_t = encp.tile([64, NI, 2, 128], BF16)

        # the opt half of enc is complete after step 63, so opt_q projection
        # and the oq transposes drain into the PE-idle slots of main-GRU
        # steps 64..127 (one thunk per step, emitted inline)
        pending = []
        for jg in range(2):
            for t0 in range(0, LO, 32):
                pending.append(lambda jg=jg, t0=t0: kq_chunk(
                    optqT, wq, jg, t0, min(32, LO - t0), NI, BL, NBM))
        for i in range(NI):
            for jg in range(2):
                pending.append(lambda i=i, jg=jg: bf_transpose(
                    oq_t[:, i, jg, :], optqT[:, jg, :, i], 128, 64))

        for t in range(LC):
            gru_step(t, whr, xpu, hm, NBM, 0, store_main)
            if t >= LO and pending:
                pending.pop(0)()
        while pending:
            pending.pop(0)()

        # ctx_key needs the full context encoding
        for jg in range(2):
            kq_chunk(ctxkT, wk, jg, 0, LC, BL, 0, BL)
        for b in range(BL):
            for jg in range(2):
                bf_transpose(ck_t[:, b, jg, :], ctxkT[:, jg, :, b], 128, 128)
        ctxk_cb = [[ck_t[:, b, jg, :] for jg in range(2)] for b in range(BL)]
        oq_all = [[oq_t[:, i, jg, :] for jg in range(2)] for i in range(NI)]

        # ======== Phase 4: attention per (b, opt) ========
        # |e| <= sum|v| ~ 8, so exp() is safe in fp32 without max-subtraction.
        # Both softmaxes share one exp(e): P1 = eu/rowsum (softmax over q,
        # free axis), P2 = eu/colsum (softmax over c, partition axis; the
        # colsum comes from a ones-row matmul and normalization is folded
        # into the aggregation post-scale).
        actxT = encp.tile([128, 2, NI, LC], BF16)
        aoptT = encp.tile([128, 2, NI, LO], BF16)
        QCH = 8
        for b in range(BL):
            for o in range(NOPT):
                i = b * NOPT + o
                e_ps = psum_e.tile([128, LO], F32, tag="e")
                for q0 in range(0, LO, QCH):
                    sts = []
                    for jg in range(2):
                        # all-16-bit s-build (DVE 2x perf mode); fp16 keeps
                        # the energy quantization error small
                        st = spool.tile([128, QCH, LC], F16, tag=f"s{jg}")
                        eng = nc.vector if (q0 // QCH + jg) % 2 == 0 else nc.gpsimd
                        eng.tensor_tensor(
                            st[:],
                            optqT[:, jg, q0:q0 + QCH, i:i + 1]
                                .broadcast_to([128, QCH, LC]),
                            ctxkT[:, jg, None, :, b]
                                .broadcast_to([128, QCH, LC]),
                            ALU.add)
                        nc.scalar.activation(st[:], st[:], AF.Tanh)
                        sts.append(st)
                    for q in range(QCH):
                        for jg in range(2):
                            nc.tensor.matmul(
                                e_ps[:, q0 + q:q0 + q + 1],
                                sts[jg][:, q, :], vsb[:, jg:jg + 1],
                                start=(jg == 0), stop=(jg == 1))
                # shared unnormalized exp(e) [c, q]
                eu = small.tile([128, LO], BF16, tag="eu")
                nc.scalar.activation(eu[:], e_ps[:], AF.Exp)
                # P1: softmax over q (free axis)
                sm = small.tile([128, 1], F32, tag="sm")
                nc.vector.tensor_reduce(sm[:], eu[:],
                                        axis=mybir.AxisListType.X, op=ALU.add)
                nc.vector.reciprocal(sm[:], sm[:])
                p1 = small.tile([128, LO], BF16, tag="p1")
                nc.vector.tensor_scalar_mul(p1[:], eu[:], sm[:])
                pt1 = psg.tile([128, 512], BF16, tag="psbf", name="pst_bf")
                nc.tensor.transpose(pt1[:64, :128], p1[:], ident[:])
                p1t = small.tile([64, 128], BF16, tag="p1tb")
                nc.vector.tensor_copy(p1t[:], pt1[:64, :128])
                # P2 colsum over c, replicated across partitions by a
                # ones-matrix matmul; normalize after aggregation
                s2_ps = ps_tile([128, 512])
                nc.tensor.matmul(s2_ps[:, :LO], ones_bf[:], eu[:],
                                 start=True, stop=True)
                r2 = small.tile([128, LO], F32, tag="r2")
                nc.vector.reciprocal(r2[:], s2_ps[:, :LO])
                for jg in range(2):
                    ac_ps = ps_tile([128, 512])
                    nc.tensor.matmul(ac_ps[:, :128], oq_all[i][jg], p1t[:],
                                     start=True, stop=True)
                    nc.scalar.copy(actxT[:, jg, i, :], ac_ps[:, :128])
                    ao_ps = ps_tile([128, 512])
                    nc.tensor.matmul(ao_ps[:, :64], ctxk_cb[b][jg], eu[:],
                                     start=True, stop=True)
                    nc.vector.tensor_tensor(
                        aoptT[:, jg, i, :], ao_ps[:, :64], r2[:], ALU.mult)

        # ======== Phase 5: att GRU input projections ========
        # Reuses xpu (main-GRU xp is dead): cols [0:NI) = actx (valid all t,
        # both dirs), cols [NI:NBA) = aopt (dir0 at t in [0,64), dir1 at t in
        # [64,128), so the uniform bwd index 127-t_f reads aopt time 63-t_f).
        # The xp bias is applied by the psum->sbuf copy (per-partition bias
        # add), not a matmul; copies round-robin across SE/VE/Pool.
        nc.gpsimd.memset(xpu[:, 0, LO:, :, NI:], 0.0)
        nc.gpsimd.memset(xpu[:, 1, :LO, :, NI:], 0.0)
        cp_eng = [0]

        def copy_bias(dst, src, bias_ap):
            e = cp_eng[0] = (cp_eng[0] + 1) % 2
            if e == 0:
                nc.scalar.activation(dst, src, AF.Identity, bias=bias_ap)
            else:
                nc.vector.tensor_scalar(dst, src, bias_ap, None, op0=ALU.add)

        def proj_att_chunk(src, dd, jg, t0, tch, T, cl, ch, tb1):
            tb = tb1 if dd == 1 else 0
            tw = min(tch, T - t0)
            cw = tw * NI
            pt = ps_tile([128, 512])
            for k in range(2):
                nc.tensor.matmul(
                    pt[:, :cw],
                    wia[:, dd, k, jg * 128:(jg + 1) * 128],
                    src[:, k, t0:t0 + tw, :],
                    start=(k == 0), stop=(k == 1))
            copy_bias(
                xpu[:, dd, tb + t0:tb + t0 + tw, jg, cl:ch],
                pt[:, :cw], biasa[:, dd, jg:jg + 1])

        # transposed views [128, k, t, i] of actxT/aoptT ([128, jg, i, t])
        acv = actxT[:].transpose([0, 1, 3, 2])
        aov = aoptT[:].transpose([0, 1, 3, 2])
        # outside-in: steps 0..31 of the att GRU only touch t in [0,32) and
        # [96,128), so the aopt and end actx chunks are projected up front
        # while the middle actx chunks drain into the first GRU steps
        for t0 in (0, 32):
            for dd in range(2):
                for jg in range(6):
                    proj_att_chunk(aov, dd, jg, t0, 32, LO, NI, NBA, LO)
        for t0 in (0, 96):
            for dd in range(2):
                for jg in range(6):
                    proj_att_chunk(acv, dd, jg, t0, 32, LC, 0, NI, 0)
        pending2 = []
        for t0 in (32, 64):
            for dd in range(2):
                for jg in range(6):
                    pending2.append(lambda t0=t0, dd=dd, jg=jg: proj_att_chunk(
                        acv, dd, jg, t0, 32, LC, 0, NI, 0))

        # ======== Phase 6: att GRU recurrence with mean accumulation ========
        ha = hpool.tile([128, 2, 2, NBA], BF16, tag="h")
        nc.vector.memset(ha[:], 0.0)
        acc = encp.tile([128, 2, 2, NBA], F32)
        nc.vector.memset(acc[:], 0.0)

        def store_att(dd, t_f, hst):
            if t_f < LO:
                nc.vector.tensor_tensor(acc[:, dd], acc[:, dd],
                                        hst[:, dd, :, :], ALU.add)
            else:
                nc.vector.tensor_tensor(acc[:, dd, :, 0:NI],
                                        acc[:, dd, :, 0:NI],
                                        hst[:, dd, :, 0:NI], ALU.add)

        for t in range(LC):
            gru_step(t, wha, xpu, ha, NBA, 1, store_att)
            if pending2:
                pending2.pop(0)()

        # ======== Phase 7: cosine similarity ========
        # means: acc cols [0:NI) = a_ctx side (per 1/LC), [NI:) = a_opt side
        # (per 1/LO); the 1/LC / 1/LO scales cancel in the cosine except as
        # a joint scale, but keep them for exactness.
        acc_c = acc[:, :, :, 0:NI]
        acc_o = acc[:, :, :, NI:NBA]
        nc.vector.tensor_scalar_mul(acc_c, acc_c, 1.0 / LC)
        nc.vector.tensor_scalar_mul(acc_o, acc_o, 1.0 / LO)
        prod = small.tile([128, 2, 2, NI], F32, tag="prod")
        dots_ps = psg.tile([1, 3, 4, NI], F32, tag="ps")
        nc.vector.tensor_tensor(prod[:], acc_c, acc_o, ALU.mult)
        nc.tensor.matmul(dots_ps[:, 0], ones128[:], prod[:],
                         start=True, stop=True)
        nc.vector.tensor_tensor(prod[:], acc_c, acc_c, ALU.mult)
        nc.tensor.matmul(dots_ps[:, 1], ones128[:], prod[:],
                         start=True, stop=True)
        nc.vector.tensor_tensor(prod[:], acc_o, acc_o, ALU.mult)
        nc.tensor.matmul(dots_ps[:, 2], ones128[:], prod[:],
                         start=True, stop=True)
        red = small.tile([1, 3, NI], F32, tag="red")
        nc.vector.tensor_reduce(red[:], dots_ps[:].transpose([0, 1, 3, 2]),
                                axis=mybir.AxisListType.X, op=ALU.add)
        nrm = small.tile([1, NI], F32, tag="nrm")
        nc.vector.tensor_tensor(nrm[:], red[:, 1, :], red[:, 2, :], ALU.mult)
        nc.vector.tensor_scalar_max(nrm[:], nrm[:], 1e-30)
        nc.scalar.activation(nrm[:], nrm[:], AF.Sqrt)
        nc.vector.reciprocal(nrm[:], nrm[:])
        cos = small.tile([1, NI], F32, tag="cos")
        nc.vector.tensor_tensor(cos[:], red[:, 0, :], nrm[:], ALU.mult)
        nc.sync.dma_start(d["out"].ap(), cos[:])


def _prep_inputs(inputs):
    ctx = np.asarray(inputs["context"], np.float32)
    opts = np.asarray(inputs["options"], np.float32)

    def gru_w(pre):
        out = {}
        for dd, sfx in enumerate(("f", "b")):
            out[dd] = {k: np.asarray(inputs[f"{pre}_{k}_{sfx}"], np.float32)
                       for k in ("Wi", "Wh", "bi", "bh")}
        return out

    rnn, att = gru_w("rnn"), gru_w("att")
    Wk = np.asarray(inputs["Wk"], np.float32)
    Wq = np.asarray(inputs["Wq"], np.float32)
    v = np.asarray(inputs["v_energy"], np.float32)

    def wi_pack(g, ein):
        out = np.zeros((2, 3, 128, H3), np.float32)
        for dd in range(2):
            bias = g[dd]["bi"].copy()
            bias[:2 * H] += g[dd]["bh"][:2 * H]
            m = np.zeros((3 * 128, H3), np.float32)
            m[:ein] = g[dd]["Wi"].T
            m[ein] = bias
            out[dd] = m.reshape(3, 128, H3)
        return out.astype(bf)

    def wh_pack(g):
        out = np.zeros((2, 2, 128, H3), np.float32)
        for dd in range(2):
            out[dd] = g[dd]["Wh"].T.reshape(2, 128, H3)
        return out.astype(bf)

    def bhn_pack(g):
        out = np.zeros((2, 2, 128), np.float32)
        for dd in range(2):
            out[dd, 0] = g[dd]["bh"][2 * H:2 * H + 128]
            out[dd, 1] = g[dd]["bh"][2 * H + 128:]
        return out

    def biasa_pack(g):
        # xp bias for the att GRU: bi with bh folded in for the r,z gates,
        # laid out [h-partition, dir, gate-block]
        out = np.zeros((128, 2, 6), np.float32)
        for dd in range(2):
            bias = g[dd]["bi"].copy()
            bias[:2 * H] += g[dd]["bh"][:2 * H]
            out[:, dd, :] = bias.reshape(6, 128).T
        return out

    shared = {
        "wir": wi_pack(rnn, E), "whr": wh_pack(rnn),
        "wia": wi_pack(att, H), "wha": wh_pack(att),
        "wk": np.ascontiguousarray(Wk.T.reshape(4, 128, H).astype(bf)),
        "wq": np.ascontiguousarray(Wq.T.reshape(4, 128, H).astype(bf)),
        "bhnrow": np.ascontiguousarray(
            np.stack([bhn_pack(rnn), bhn_pack(att)]).astype(bf)),
        "biasa": np.ascontiguousarray(biasa_pack(att)),
        "v": np.ascontiguousarray(v.reshape(2, 128).T.astype(np.float16)),
    }

    in_maps = []
    for c in range(NCORES):
        bs = slice(c * BL, (c + 1) * BL)
        xa = np.zeros((BL, LC, 3 * 128), np.float32)
        xa[:, :, :E] = ctx[bs]
        xa[:, :, E] = 1.0
        xb = np.zeros((NI, LO, 3 * 128), np.float32)
        xb[:, :, :E] = opts[bs].reshape(NI, LO, E)
        xb[:, :, E] = 1.0
        m = dict(shared)
        m["xtc"] = np.ascontiguousarray(
            xa.transpose(2, 1, 0).reshape(3, 128, LC * BL).astype(bf))
        m["xto"] = np.ascontiguousarray(
            xb.transpose(2, 1, 0).reshape(3, 128, LO * NI).astype(bf))
        in_maps.append(m)
    return in_maps


def kernel(**inputs):
    if "nc" not in _CACHE:
        _CACHE["nc"] = _build()
    nc = _CACHE["nc"]
    in_maps = _prep_inputs(inputs)
    res = bass_utils.run_bass_kernel_spmd(nc, in_maps,
                                          core_ids=list(range(NCORES)))
    _CACHE["last_exec_ns"] = res.exec_time_ns
    _CACHE["last_res"] = res
    logits = np.concatenate(
        [np.asarray(res.results[c]["out"], np.float32).reshape(BL, NOPT)
         for c in range(NCORES)], axis=0)
    x = logits - logits.max(axis=1, keepdims=True)
    ex = np.exp(x)
    return (ex / ex.sum(axis=1, keepdims=True)).astype(np.float32)


if __name__ == "__main__":
    _build()
    print("build+compile OK")

